# revision 16
# baseline (speedup 1.0000x reference)
"""CNLinkPredictor Trainium2 kernel.

Per-edge common-neighbor link predictor over 8 NeuronCores (data-parallel over
the 32768 target edges, 4096 per core).

Device pipeline per core:
  1. dma_gather adjacency rows adj[i_e], adj[j_e] and feature rows x[i_e], x[j_e]
     (edges laid out 32-per-partition: edge-slot e_loc = p*32 + c).
  2. cm via 64 int16 "rotation" compares on DVE:
     cm[e,a] = OR_b (ni[e,a] == nj[e,b]).
  3. Sparse compaction: per-partition rank (prefix scan) + local_scatter of the
     winning (edge, node) pairs into <=128 slots per partition.
  4. dma_gather of ONLY the winners' x rows; a per-window "selection matmul"
     (gathered rows stationary, 0/1 edge-indicator moving) sums them into
     xcn^T [64, 4096] -- this dedups, masks, and transposes in one PE pass.
  5. Dense f32 MLP stack with features/hidden in partitions, edges streaming in
     the free dimension; y [1, 4096] DMA'd out.
"""

import numpy as np

N, D, E_TOT, F, H = 10000, 64, 32768, 64, 256
NCORES = 8
E_LOC = E_TOT // NCORES      # 4096
P = 128
WEDGE = 32                   # edges per window (= per partition)
NWIN = 128                   # windows per core (= partitions)
KSLOT = 128                  # winner slots per window
NT = WEDGE * D               # 2048 dense (edge,slot) pairs per partition

_CACHE = {}


def _build(stop=5):
    import concourse.bacc as bacc
    import concourse.mybir as mybir
    import concourse.tile as tile
    from concourse.masks import make_identity

    dt = mybir.dt
    op = mybir.AluOpType
    act = mybir.ActivationFunctionType

    nc = bacc.Bacc(None, dynamic_dma_scratch_size=32768, num_swdge_queues=4)
    nm = {}
    with tile.TileContext(nc) as tc:
        with tc.tile_pool(name="dram", bufs=1, space="DRAM") as dram, \
             tc.tile_pool(name="keep", bufs=1) as keep, \
             tc.tile_pool(name="ps", bufs=2, space="PSUM") as ps:

            # ---------------- DRAM I/O ----------------
            adjx = dram.tile([N, D + F], dt.int32, kind="ExternalInput")
            xrows = dram.tile([N, F], dt.float32, kind="ExternalInput")
            idx_i = dram.tile([128, E_LOC // 16], dt.int16, kind="ExternalInput")
            idx_j = dram.tile([128, E_LOC // 16], dt.int16, kind="ExternalInput")
            w_xij1 = dram.tile([64, 256], dt.float32, kind="ExternalInput")
            w_xij2 = dram.tile([128, 512], dt.float32, kind="ExternalInput")
            w_cn1 = dram.tile([64, 256], dt.float32, kind="ExternalInput")
            w_cn2 = dram.tile([128, 512], dt.float32, kind="ExternalInput")
            w_cn3 = dram.tile([128, 512], dt.float32, kind="ExternalInput")
            w_lin1 = dram.tile([128, 512], dt.float32, kind="ExternalInput")
            w_lin2 = dram.tile([128, 2], dt.float32, kind="ExternalInput")
            biases = dram.tile([128, 14], dt.float32, kind="ExternalInput")
            # bias cols: xij1(2) xij2(2) cn1(2) cn2(2) cn3(2) lin1(2) [lin2, beta]
            y_out = dram.tile([1, E_LOC], dt.float32, kind="ExternalOutput")
            cwd = dram.tile([16, NWIN * KSLOT // 16], dt.int16)

            nm.update(adjx=adjx.name, xrows=xrows.name, idx_i=idx_i.name,
                      idx_j=idx_j.name, w_xij1=w_xij1.name, w_xij2=w_xij2.name,
                      w_cn1=w_cn1.name, w_cn2=w_cn2.name, w_cn3=w_cn3.name,
                      w_lin1=w_lin1.name, w_lin2=w_lin2.name, biases=biases.name,
                      y_out=y_out.name)

            # -------- long-lived tiles --------
            IDENT = keep.tile([P, P], dt.float32)
            make_identity(nc, IDENT[:])
            WX1 = keep.tile([64, 256], dt.float32); nc.sync.dma_start(WX1[:], w_xij1[:])
            WX2 = keep.tile([128, 512], dt.float32); nc.sync.dma_start(WX2[:], w_xij2[:])
            WC1 = keep.tile([64, 256], dt.float32); nc.sync.dma_start(WC1[:], w_cn1[:])
            WC2 = keep.tile([128, 512], dt.float32); nc.sync.dma_start(WC2[:], w_cn2[:])
            WC3 = keep.tile([128, 512], dt.float32); nc.sync.dma_start(WC3[:], w_cn3[:])
            WL1 = keep.tile([128, 512], dt.float32); nc.sync.dma_start(WL1[:], w_lin1[:])
            WL2 = keep.tile([128, 2], dt.float32); nc.sync.dma_start(WL2[:], w_lin2[:])
            BIA = keep.tile([128, 14], dt.float32); nc.sync.dma_start(BIA[:], biases[:])
            XCN = keep.tile([64, E_LOC], dt.float32)
            XIJT = keep.tile([64, E_LOC], dt.float32)
            IDXG = keep.tile([128, NWIN * KSLOT // 16], dt.int16)
            CELT = keep.tile([P, NWIN], dt.float32)
            YSB = keep.tile([1, E_LOC], dt.float32)
            if stop < 5:
                nc.vector.memset(YSB[:], 0.0)
                nc.sync.dma_start(y_out[:], YSB[:])

            # ======== stage A: gathers, xij^T, cm, compaction ========
            with tc.tile_pool(name="sa", bufs=1) as sa, \
                 tc.tile_pool(name="wk", bufs=2) as wk:
                IDXI = sa.tile([128, E_LOC // 16], dt.int16)
                nc.sync.dma_start(IDXI[:], idx_i[:])
                IDXJ = sa.tile([128, E_LOC // 16], dt.int16)
                nc.sync.dma_start(IDXJ[:], idx_j[:])

                AXI = sa.tile([P, WEDGE, D + F], dt.int32)
                for ch in range(4):
                    nc.gpsimd.dma_gather(AXI[:, ch * 8:(ch + 1) * 8, :], adjx[:],
                                         IDXI[:, ch * 64:(ch + 1) * 64],
                                         num_idxs=1024, num_idxs_reg=1024,
                                         elem_size=D + F, queue_num=ch)
                AXJ = sa.tile([P, WEDGE, D + F], dt.int32)
                for ch in range(4):
                    nc.gpsimd.dma_gather(AXJ[:, ch * 8:(ch + 1) * 8, :], adjx[:],
                                         IDXJ[:, ch * 64:(ch + 1) * 64],
                                         num_idxs=1024, num_idxs_reg=1024,
                                         elem_size=D + F, queue_num=ch)
                NI32 = AXI[:, :, 0:D]
                NJ32 = AXJ[:, :, 0:D]
                XI = AXI[:, :, D:D + F].bitcast(dt.float32)
                XJ = AXJ[:, :, D:D + F].bitcast(dt.float32)

                # xij^T via PE transposes (early, frees XI/XJ)
                XIJ = sa.tile([P, WEDGE, F], dt.float32)
                nc.vector.tensor_tensor(out=XIJ[:], in0=XI, in1=XJ, op=op.mult)
                xijt_v = XIJT[:].rearrange("f (p bk q) -> f bk q p", bk=8, q=4)
                for bank in range(8):
                    pt = ps.tile([64, 4, 128], dt.float32, tag="sel", name=f"ptr{bank}")
                    for q in range(4):
                        c = bank * 4 + q
                        nc.tensor.matmul(pt[:, q, :], lhsT=XIJ[:, c, :], rhs=IDENT[:],
                                         start=True, stop=True)
                    nc.scalar.activation(xijt_v[:, bank], pt[:], act.Copy)

                if stop >= 2:
                    # int16 neighbor ids
                    NI16 = sa.tile([P, WEDGE, D], dt.int16)
                    nc.vector.tensor_copy(NI16[:], NI32)
                    NIP1 = sa.tile([P, WEDGE, D], dt.int32)
                    nc.vector.tensor_scalar(out=NIP1[:], in0=NI32,
                                            scalar1=1, scalar2=None, op0=op.add)
                    NIP116 = sa.tile([P, WEDGE, D], dt.int16)
                    nc.vector.tensor_copy(NIP116[:], NIP1[:])
                    NJd = sa.tile([P, WEDGE, 2 * D], dt.int16)
                    nc.vector.tensor_copy(NJd[:, :, 0:D], NJ32)
                    nc.vector.tensor_copy(NJd[:, :, D:2 * D], NJd[:, :, 0:D])

                    # cm rotation loop
                    ACC = sa.tile([P, WEDGE, D], dt.int16)
                    nc.vector.memset(ACC[:], 0)
                    for r in range(D):
                        EQ = wk.tile([P, WEDGE, D], dt.int16, tag="eq", name=f"eq{r}")
                        nc.vector.tensor_tensor(out=EQ[:], in0=NI16[:],
                                                in1=NJd[:, :, r:r + D], op=op.is_equal)
                        nc.vector.tensor_tensor(out=ACC[:], in0=ACC[:], in1=EQ[:],
                                                op=op.max)

                if stop >= 3:
                    # compaction
                    CMF = sa.tile([P, NT], dt.float32)
                    nc.vector.tensor_copy(CMF[:], ACC[:].rearrange("p a b -> p (a b)"))
                    ONES = sa.tile([P, NT], dt.float32)
                    nc.vector.memset(ONES[:], 1.0)
                    RANK = sa.tile([P, NT], dt.float32)
                    nc.vector.tensor_tensor_scan(RANK[:], ONES[:], CMF[:], 0.0,
                                                 op.mult, op.add)
                    SLOTF = sa.tile([P, NT], dt.float32)
                    nc.vector.tensor_tensor(out=SLOTF[:], in0=RANK[:], in1=CMF[:], op=op.mult)
                    nc.vector.tensor_scalar(out=SLOTF[:], in0=SLOTF[:], scalar1=-1.0,
                                            scalar2=None, op0=op.add)
                    nc.vector.tensor_scalar(out=SLOTF[:], in0=SLOTF[:],
                                            scalar1=float(KSLOT - 1), scalar2=None, op0=op.min)
                    SLOT = sa.tile([P, NT], dt.int16)
                    nc.vector.tensor_copy(SLOT[:], SLOTF[:])
                    CEL16 = sa.tile([P, NT], dt.int16)
                    nc.gpsimd.iota(CEL16[:], pattern=[[1, WEDGE], [0, D]], base=1,
                                   channel_multiplier=0)
                    CW = sa.tile([P, KSLOT], dt.int16)
                    nc.gpsimd.local_scatter(CW[:], NIP116[:].rearrange("p a b -> p (a b)"),
                                            SLOT[:], channels=P,
                                            num_elems=KSLOT, num_idxs=NT)
                    CEL = sa.tile([P, KSLOT], dt.int16)
                    nc.gpsimd.local_scatter(CEL[:], CEL16[:], SLOT[:], channels=P,
                                            num_elems=KSLOT, num_idxs=NT)
                    CWm1 = sa.tile([P, KSLOT], dt.int16)
                    nc.vector.tensor_scalar(out=CWm1[:], in0=CW[:], scalar1=-1,
                                            scalar2=None, op0=op.add)
                    nc.vector.tensor_scalar(out=CWm1[:], in0=CWm1[:], scalar1=0,
                                            scalar2=None, op0=op.max)
                    # write CWm1 to DRAM already in wrapped [16, 1024] layout:
                    # dst flat(p16, w*8+kh) <- CWm1[w, k=kh*16+p16]
                    cwd_wv = cwd[:].rearrange("p (w kh) -> w kh p", kh=8)
                    nc.sync.dma_start(cwd_wv, CWm1[:].rearrange("w (kh p) -> w kh p", p=16))
                    for g in range(8):
                        nc.sync.dma_start(IDXG[16 * g:16 * (g + 1), :], cwd[:])

                    CELF = sa.tile([P, KSLOT], dt.float32)
                    nc.vector.tensor_copy(CELF[:], CEL[:])
                    CELT_ps = ps.tile([P, P], dt.float32, tag="tr", bufs=1)
                    nc.tensor.transpose(CELT_ps[:], CELF[:], IDENT[:])
                    nc.vector.tensor_copy(CELT[:], CELT_ps[:])

            # ======== stage B: winner gather + selection matmuls ========
            if stop >= 4:
                with tc.tile_pool(name="sbp", bufs=1) as sbp:
                    XG = sbp.tile([P, NWIN, F], dt.float32)
                    for ch in range(16):
                        nc.gpsimd.dma_gather(XG[:, ch * 8:(ch + 1) * 8, :], xrows[:],
                                             IDXG[:, ch * 64:(ch + 1) * 64],
                                             num_idxs=1024, num_idxs_reg=1024,
                                             elem_size=F, queue_num=ch % 4)
                    INDN = sbp.tile([P, WEDGE], dt.int32)
                    nc.gpsimd.iota(INDN[:], pattern=[[1, WEDGE]], base=1, channel_multiplier=0)
                    INDNF = sbp.tile([P, WEDGE], dt.float32)
                    nc.vector.tensor_copy(INDNF[:], INDN[:])
                    IND = sbp.tile([P, NWIN, WEDGE], dt.float32)
                    nc.vector.tensor_tensor(out=IND[:],
                                            in0=CELT[:].to_broadcast([P, NWIN, WEDGE]),
                                            in1=INDNF[:].unsqueeze(1).to_broadcast([P, NWIN, WEDGE]),
                                            op=op.is_equal)
                    for bank in range(8):
                        pj = ps.tile([64, 512], dt.float32, tag="sel", name=f"pj{bank}")
                        for w16 in range(16):
                            w = bank * 16 + w16
                            nc.tensor.matmul(pj[:, w16 * 32:(w16 + 1) * 32],
                                             lhsT=XG[:, w, :], rhs=IND[:, w, :],
                                             start=True, stop=True)
                        nc.scalar.activation(XCN[:, bank * 512:(bank + 1) * 512], pj[:], act.Copy)

            # ======== stage C: MLP stack ========
            if stop >= 5:
                with tc.tile_pool(name="hw", bufs=4) as hw:
                    ZER = hw.tile([P, 512], dt.float32, tag="zer", bufs=1)
                    nc.vector.memset(ZER[:], 0.0)

                    def layer(src, wt, bcol, kchunks, relu_, tag):
                        fn = act.Relu if relu_ else act.Identity
                        dst = [hw.tile([P, E_LOC], dt.float32, tag=tag,
                                       name=f"{tag}_{bcol}_{m}", bufs=2 if tag != "hwk" else 4)
                               for m in range(2)]
                        for n in range(8):
                            nsl = slice(n * 512, (n + 1) * 512)
                            for m in range(2):
                                pt = ps.tile([P, 512], dt.float32, tag="mlp",
                                             name=f"mlp_{bcol}_{n}_{m}", bufs=4)
                                for kc in range(kchunks):
                                    if kchunks == 1:
                                        lhs = wt[:, m * 128:(m + 1) * 128]
                                    else:
                                        lhs = wt[:, kc * 256 + m * 128: kc * 256 + (m + 1) * 128]
                                    nc.tensor.matmul(pt[:], lhsT=lhs, rhs=src[kc][:, nsl],
                                                     start=(kc == 0), stop=(kc == kchunks - 1))
                                if (n + m) % 2 == 0:
                                    nc.scalar.activation(dst[m][:, nsl], pt[:], fn,
                                                         bias=BIA[:, bcol + m: bcol + m + 1])
                                elif relu_:
                                    nc.vector.scalar_tensor_tensor(
                                        out=dst[m][:, nsl], in0=pt[:],
                                        scalar=BIA[:, bcol + m: bcol + m + 1],
                                        in1=ZER[:], op0=op.add, op1=op.max)
                                else:
                                    nc.vector.tensor_scalar(
                                        out=dst[m][:, nsl], in0=pt[:],
                                        scalar1=BIA[:, bcol + m: bcol + m + 1],
                                        scalar2=None, op0=op.add)
                        return dst

                    H1 = layer([XIJT], WX1, 0, 1, True, "h1")
                    HIJ = layer(H1, WX2, 2, 2, False, "hij")
                    C1 = layer([XCN], WC1, 4, 1, True, "hwk")
                    C2 = layer(C1, WC2, 6, 2, True, "hwk")
                    HCN = layer(C2, WC3, 8, 2, False, "hwk")
                    Z = [hw.tile([P, E_LOC], dt.float32, tag="hwk", name=f"z_{m}")
                         for m in range(2)]
                    for m in range(2):
                        nc.vector.scalar_tensor_tensor(out=Z[m][:], in0=HCN[m][:],
                                                       scalar=BIA[:, 13:14], in1=HIJ[m][:],
                                                       op0=op.mult, op1=op.add)
                    G = layer(Z, WL1, 10, 2, True, "hwk")
                    for n in range(8):
                        nsl = slice(n * 512, (n + 1) * 512)
                        pt = ps.tile([1, 512], dt.float32, tag="y", name=f"y_{n}", bufs=1)
                        for kc in range(2):
                            nc.tensor.matmul(pt[:], lhsT=WL2[:, kc:kc + 1], rhs=G[kc][:, nsl],
                                             start=(kc == 0), stop=(kc == 1))
                        nc.scalar.activation(YSB[:, nsl], pt[:], act.Identity,
                                             bias=BIA[:1, 12:13])
                nc.sync.dma_start(y_out[:], YSB[:])

    nc.compile()
    return nc, nm


def _wrap_idx(v):
    """[E_LOC] int array -> [32, E_LOC//16] int16 wrapped + replicated."""
    w = v.reshape(E_LOC // 16, 16).T.astype(np.int16)
    return np.tile(w, (8, 1))


def _pack_w2(w):
    """[256, X] -> [128, 2*X] with k-chunk kc at cols [kc*X, (kc+1)*X)."""
    X = w.shape[1]
    return np.ascontiguousarray(
        w.reshape(2, 128, X).transpose(1, 0, 2).reshape(128, 2 * X)).astype(np.float32)


def kernel(_profile=False, **inputs):
    from concourse.bass_utils import run_bass_kernel_spmd

    x = np.asarray(inputs["x"], np.float32)
    adj = np.asarray(inputs["adj_nbr"], np.int64)
    tar = np.asarray(inputs["tar_ei"], np.int64)
    beta = np.asarray(inputs["beta"], np.float32).reshape(-1)[0]

    if "nc" not in _CACHE:
        _CACHE["nc"], _CACHE["nm"] = _build()
    nc, nm = _CACHE["nc"], _CACHE["nm"]

    adj32 = adj.astype(np.int32)
    adjx = np.ascontiguousarray(np.concatenate([adj32, x.view(np.int32)], axis=1))
    xr = np.ascontiguousarray(x)

    # spread i==j edges so no window gets two of them (64 winners each)
    ti, tj = tar[0], tar[1]
    iijj = np.where(ti == tj)[0]
    perm = np.arange(E_TOT)
    if len(iijj):
        assert len(iijj) * 32 <= E_TOT
        pos = np.arange(len(iijj)) * 32
        mask = np.ones(E_TOT, bool)
        mask[pos] = False
        rest = np.setdiff1d(perm, iijj, assume_unique=True)
        perm = np.empty(E_TOT, np.int64)
        perm[pos] = iijj
        perm[mask] = rest

    # biases packing: [128, 14]
    def b2(b):
        return np.asarray(b, np.float32).reshape(2, 128).T
    BIA = np.zeros((128, 14), np.float32)
    BIA[:, 0:2] = b2(inputs["xij_b1"]); BIA[:, 2:4] = b2(inputs["xij_b2"])
    BIA[:, 4:6] = b2(inputs["xcn_b1"]); BIA[:, 6:8] = b2(inputs["xcn_b2"])
    BIA[:, 8:10] = b2(inputs["xcn_b3"]); BIA[:, 10:12] = b2(inputs["lin_b1"])
    BIA[:, 12] = np.float32(np.asarray(inputs["lin_b2"], np.float32).reshape(-1)[0])
    BIA[:, 13] = beta

    shared = {
        nm["adjx"]: adjx, nm["xrows"]: xr,
        nm["w_xij1"]: np.ascontiguousarray(np.asarray(inputs["xij_w1"], np.float32)),
        nm["w_xij2"]: _pack_w2(np.asarray(inputs["xij_w2"], np.float32)),
        nm["w_cn1"]: np.ascontiguousarray(np.asarray(inputs["xcn_w1"], np.float32)),
        nm["w_cn2"]: _pack_w2(np.asarray(inputs["xcn_w2"], np.float32)),
        nm["w_cn3"]: _pack_w2(np.asarray(inputs["xcn_w3"], np.float32)),
        nm["w_lin1"]: _pack_w2(np.asarray(inputs["lin_w1"], np.float32)),
        nm["w_lin2"]: _pack_w2(np.asarray(inputs["lin_w2"], np.float32)),
        nm["biases"]: BIA,
    }

    t = np.arange(E_LOC)
    e_loc_of_t = (t % 128) * WEDGE + t // 128
    in_maps = []
    for k in range(NCORES):
        ge = perm[k * E_LOC + e_loc_of_t]
        m = dict(shared)
        m[nm["idx_i"]] = _wrap_idx(ti[ge])
        m[nm["idx_j"]] = _wrap_idx(tj[ge])
        in_maps.append(m)

    kw = dict(trace=True, trace_cores=[0]) if _profile else {}
    res = run_bass_kernel_spmd(nc, in_maps, core_ids=list(range(NCORES)), **kw)
    _CACHE["last"] = res
    y = np.empty(E_TOT, np.float32)
    for k in range(NCORES):
        y[perm[k * E_LOC + np.arange(E_LOC)]] = res.results[k][nm["y_out"]].reshape(-1)
    return y.reshape(E_TOT, 1)


# revision 19
# speedup vs baseline: 1.0026x; 1.0026x over previous
"""CNLinkPredictor Trainium2 kernel.

Per-edge common-neighbor link predictor over 8 NeuronCores (data-parallel over
the 32768 target edges, 4096 per core).

Device pipeline per core:
  1. dma_gather adjacency rows adj[i_e], adj[j_e] and feature rows x[i_e], x[j_e]
     (edges laid out 32-per-partition: edge-slot e_loc = p*32 + c).
  2. cm via 64 int16 "rotation" compares on DVE:
     cm[e,a] = OR_b (ni[e,a] == nj[e,b]).
  3. Sparse compaction: per-partition rank (prefix scan) + local_scatter of the
     winning (edge, node) pairs into <=128 slots per partition.
  4. dma_gather of ONLY the winners' x rows; a per-window "selection matmul"
     (gathered rows stationary, 0/1 edge-indicator moving) sums them into
     xcn^T [64, 4096] -- this dedups, masks, and transposes in one PE pass.
  5. Dense f32 MLP stack with features/hidden in partitions, edges streaming in
     the free dimension; y [1, 4096] DMA'd out.
"""

import numpy as np

N, D, E_TOT, F, H = 10000, 64, 32768, 64, 256
NCORES = 8
E_LOC = E_TOT // NCORES      # 4096
P = 128
WEDGE = 32                   # edges per window (= per partition)
NWIN = 128                   # windows per core (= partitions)
KSLOT = 128                  # winner slots per window
NT = WEDGE * D               # 2048 dense (edge,slot) pairs per partition

_CACHE = {}


def _build(stop=5):
    import concourse.bacc as bacc
    import concourse.mybir as mybir
    import concourse.tile as tile
    from concourse.masks import make_identity

    dt = mybir.dt
    op = mybir.AluOpType
    act = mybir.ActivationFunctionType

    nc = bacc.Bacc(None, dynamic_dma_scratch_size=32768, num_swdge_queues=4)
    nm = {}
    with tile.TileContext(nc) as tc:
        with tc.tile_pool(name="dram", bufs=1, space="DRAM") as dram, \
             tc.tile_pool(name="keep", bufs=1) as keep:

            # ---------------- DRAM I/O ----------------
            adjx = dram.tile([N, D + F], dt.int32, kind="ExternalInput")
            xrows = dram.tile([N, F], dt.float32, kind="ExternalInput")
            idx_i = dram.tile([128, E_LOC // 16], dt.int16, kind="ExternalInput")
            idx_j = dram.tile([128, E_LOC // 16], dt.int16, kind="ExternalInput")
            w_xij1 = dram.tile([64, 256], dt.float32, kind="ExternalInput")
            w_xij2 = dram.tile([128, 512], dt.float32, kind="ExternalInput")
            w_cn1 = dram.tile([64, 256], dt.float32, kind="ExternalInput")
            w_cn2 = dram.tile([128, 512], dt.float32, kind="ExternalInput")
            w_cn3 = dram.tile([128, 512], dt.float32, kind="ExternalInput")
            w_lin1 = dram.tile([128, 512], dt.float32, kind="ExternalInput")
            w_lin2 = dram.tile([128, 2], dt.float32, kind="ExternalInput")
            biases = dram.tile([128, 14], dt.float32, kind="ExternalInput")
            # bias cols: xij1(2) xij2(2) cn1(2) cn2(2) cn3(2) lin1(2) [lin2, beta]
            y_out = dram.tile([1, E_LOC], dt.float32, kind="ExternalOutput")
            cwd = dram.tile([16, NWIN * KSLOT // 16], dt.int16)

            nm.update(adjx=adjx.name, xrows=xrows.name, idx_i=idx_i.name,
                      idx_j=idx_j.name, w_xij1=w_xij1.name, w_xij2=w_xij2.name,
                      w_cn1=w_cn1.name, w_cn2=w_cn2.name, w_cn3=w_cn3.name,
                      w_lin1=w_lin1.name, w_lin2=w_lin2.name, biases=biases.name,
                      y_out=y_out.name)

            # -------- long-lived tiles --------
            IDENT = keep.tile([P, P], dt.float32)
            make_identity(nc, IDENT[:])
            WX1 = keep.tile([64, 256], dt.float32); nc.sync.dma_start(WX1[:], w_xij1[:])
            WX2 = keep.tile([128, 512], dt.float32); nc.sync.dma_start(WX2[:], w_xij2[:])
            WC1 = keep.tile([64, 256], dt.float32); nc.sync.dma_start(WC1[:], w_cn1[:])
            WC2 = keep.tile([128, 512], dt.float32); nc.sync.dma_start(WC2[:], w_cn2[:])
            WC3 = keep.tile([128, 512], dt.float32); nc.sync.dma_start(WC3[:], w_cn3[:])
            WL1 = keep.tile([128, 512], dt.float32); nc.sync.dma_start(WL1[:], w_lin1[:])
            WL2 = keep.tile([128, 2], dt.float32); nc.sync.dma_start(WL2[:], w_lin2[:])
            BIA = keep.tile([128, 14], dt.float32); nc.sync.dma_start(BIA[:], biases[:])
            XCN = keep.tile([64, E_LOC], dt.float32)
            XIJT = keep.tile([64, E_LOC], dt.float32)
            IDXG = keep.tile([128, NWIN * KSLOT // 16], dt.int16)
            CELT = keep.tile([P, NWIN], dt.float32)
            YSB = keep.tile([1, E_LOC], dt.float32)
            if stop < 5:
                nc.vector.memset(YSB[:], 0.0)
                nc.sync.dma_start(y_out[:], YSB[:])

            # ======== stage A: gathers, xij^T, cm, compaction ========
            with tc.tile_pool(name="sa", bufs=1) as sa, \
                 tc.tile_pool(name="wk", bufs=2) as wk, \
                 tc.tile_pool(name="ps", bufs=2, space="PSUM") as ps:
                IDXI = sa.tile([128, E_LOC // 16], dt.int16)
                nc.sync.dma_start(IDXI[:], idx_i[:])
                IDXJ = sa.tile([128, E_LOC // 16], dt.int16)
                nc.sync.dma_start(IDXJ[:], idx_j[:])

                AXI = sa.tile([P, WEDGE, D + F], dt.int32)
                for ch in range(4):
                    nc.gpsimd.dma_gather(AXI[:, ch * 8:(ch + 1) * 8, :], adjx[:],
                                         IDXI[:, ch * 64:(ch + 1) * 64],
                                         num_idxs=1024, num_idxs_reg=1024,
                                         elem_size=D + F, queue_num=ch)
                AXJ = sa.tile([P, WEDGE, D + F], dt.int32)
                for ch in range(4):
                    nc.gpsimd.dma_gather(AXJ[:, ch * 8:(ch + 1) * 8, :], adjx[:],
                                         IDXJ[:, ch * 64:(ch + 1) * 64],
                                         num_idxs=1024, num_idxs_reg=1024,
                                         elem_size=D + F, queue_num=ch)
                NI32 = AXI[:, :, 0:D]
                NJ32 = AXJ[:, :, 0:D]
                XI = AXI[:, :, D:D + F].bitcast(dt.float32)
                XJ = AXJ[:, :, D:D + F].bitcast(dt.float32)

                # xij^T via PE transposes (early, frees XI/XJ)
                XIJ = sa.tile([P, WEDGE, F], dt.float32)
                nc.vector.tensor_tensor(out=XIJ[:], in0=XI, in1=XJ, op=op.mult)
                xijt_v = XIJT[:].rearrange("f (p bk q) -> f bk q p", bk=8, q=4)
                for bank in range(8):
                    pt = ps.tile([64, 4, 128], dt.float32, tag="sel", name=f"ptr{bank}")
                    for q in range(4):
                        c = bank * 4 + q
                        nc.tensor.matmul(pt[:, q, :], lhsT=XIJ[:, c, :], rhs=IDENT[:],
                                         start=True, stop=True)
                    nc.scalar.activation(xijt_v[:, bank], pt[:], act.Copy)

                if stop >= 2:
                    # int16 neighbor ids
                    NI16 = sa.tile([P, WEDGE, D], dt.int16)
                    nc.vector.tensor_copy(NI16[:], NI32)
                    NIP1 = sa.tile([P, WEDGE, D], dt.int32)
                    nc.vector.tensor_scalar(out=NIP1[:], in0=NI32,
                                            scalar1=1, scalar2=None, op0=op.add)
                    NIP116 = sa.tile([P, WEDGE, D], dt.int16)
                    nc.vector.tensor_copy(NIP116[:], NIP1[:])
                    NJd = sa.tile([P, WEDGE, 2 * D], dt.int16)
                    nc.vector.tensor_copy(NJd[:, :, 0:D], NJ32)
                    nc.vector.tensor_copy(NJd[:, :, D:2 * D], NJd[:, :, 0:D])
                    # 4B-aligned odd-shift copy so every rotation slice hits
                    # the DVE 2x perf mode (odd element offsets fall to 1x)
                    NJe = sa.tile([P, WEDGE, 2 * D], dt.int16)
                    nc.vector.tensor_copy(NJe[:, :, 0:2 * D - 1], NJd[:, :, 1:2 * D])

                    # cm rotation loop
                    ACC = sa.tile([P, WEDGE, D], dt.int16)
                    nc.vector.memset(ACC[:], 0)
                    for r in range(D):
                        EQ = wk.tile([P, WEDGE, D], dt.int16, tag="eq", name=f"eq{r}")
                        src = NJd[:, :, r:r + D] if r % 2 == 0 \
                            else NJe[:, :, r - 1:r - 1 + D]
                        nc.vector.tensor_tensor(out=EQ[:], in0=NI16[:],
                                                in1=src, op=op.is_equal)
                        nc.vector.tensor_tensor(out=ACC[:], in0=ACC[:], in1=EQ[:],
                                                op=op.max)

                if stop >= 3:
                    # compaction
                    CMF = sa.tile([P, NT], dt.float32)
                    nc.vector.tensor_copy(CMF[:], ACC[:].rearrange("p a b -> p (a b)"))
                    ONES = sa.tile([P, NT], dt.float32)
                    nc.vector.memset(ONES[:], 1.0)
                    RANK = sa.tile([P, NT], dt.float32)
                    nc.vector.tensor_tensor_scan(RANK[:], ONES[:], CMF[:], 0.0,
                                                 op.mult, op.add)
                    SLOTF = sa.tile([P, NT], dt.float32)
                    nc.vector.tensor_tensor(out=SLOTF[:], in0=RANK[:], in1=CMF[:], op=op.mult)
                    nc.vector.tensor_scalar(out=SLOTF[:], in0=SLOTF[:], scalar1=-1.0,
                                            scalar2=None, op0=op.add)
                    nc.vector.tensor_scalar(out=SLOTF[:], in0=SLOTF[:],
                                            scalar1=float(KSLOT - 1), scalar2=None, op0=op.min)
                    SLOT = sa.tile([P, NT], dt.int16)
                    nc.vector.tensor_copy(SLOT[:], SLOTF[:])
                    CEL16 = sa.tile([P, NT], dt.int16)
                    nc.gpsimd.iota(CEL16[:], pattern=[[1, WEDGE], [0, D]], base=1,
                                   channel_multiplier=0)
                    CW = sa.tile([P, KSLOT], dt.int16)
                    nc.gpsimd.local_scatter(CW[:], NIP116[:].rearrange("p a b -> p (a b)"),
                                            SLOT[:], channels=P,
                                            num_elems=KSLOT, num_idxs=NT)
                    CEL = sa.tile([P, KSLOT], dt.int16)
                    nc.gpsimd.local_scatter(CEL[:], CEL16[:], SLOT[:], channels=P,
                                            num_elems=KSLOT, num_idxs=NT)
                    CWm1 = sa.tile([P, KSLOT], dt.int16)
                    nc.vector.tensor_scalar(out=CWm1[:], in0=CW[:], scalar1=-1,
                                            scalar2=None, op0=op.add)
                    nc.vector.tensor_scalar(out=CWm1[:], in0=CWm1[:], scalar1=0,
                                            scalar2=None, op0=op.max)
                    # write CWm1 to DRAM already in wrapped [16, 1024] layout:
                    # dst flat(p16, w*8+kh) <- CWm1[w, k=kh*16+p16]
                    cwd_wv = cwd[:].rearrange("p (w kh) -> w kh p", kh=8)
                    nc.sync.dma_start(cwd_wv, CWm1[:].rearrange("w (kh p) -> w kh p", p=16))
                    for g in range(8):
                        nc.sync.dma_start(IDXG[16 * g:16 * (g + 1), :], cwd[:])

                    CELF = sa.tile([P, KSLOT], dt.float32)
                    nc.vector.tensor_copy(CELF[:], CEL[:])
                    CELT_ps = ps.tile([P, P], dt.float32, tag="tr", bufs=1)
                    nc.tensor.transpose(CELT_ps[:], CELF[:], IDENT[:])
                    nc.vector.tensor_copy(CELT[:], CELT_ps[:])

            # ======== stage B: winner gather + selection matmuls ========
            if stop >= 4:
                with tc.tile_pool(name="sbp", bufs=1) as sbp, \
                     tc.tile_pool(name="ps", bufs=2, space="PSUM") as ps:
                    XG = sbp.tile([P, NWIN, F], dt.float32)
                    for ch in range(16):
                        nc.gpsimd.dma_gather(XG[:, ch * 8:(ch + 1) * 8, :], xrows[:],
                                             IDXG[:, ch * 64:(ch + 1) * 64],
                                             num_idxs=1024, num_idxs_reg=1024,
                                             elem_size=F, queue_num=ch % 4)
                    INDN = sbp.tile([P, WEDGE], dt.int32)
                    nc.gpsimd.iota(INDN[:], pattern=[[1, WEDGE]], base=1, channel_multiplier=0)
                    INDNF = sbp.tile([P, WEDGE], dt.float32)
                    nc.vector.tensor_copy(INDNF[:], INDN[:])
                    IND = sbp.tile([P, NWIN, WEDGE], dt.float32)
                    nc.vector.tensor_tensor(out=IND[:],
                                            in0=CELT[:].to_broadcast([P, NWIN, WEDGE]),
                                            in1=INDNF[:].unsqueeze(1).to_broadcast([P, NWIN, WEDGE]),
                                            op=op.is_equal)
                    for bank in range(8):
                        pj = ps.tile([64, 512], dt.float32, tag="sel", name=f"pj{bank}")
                        for w16 in range(16):
                            w = bank * 16 + w16
                            nc.tensor.matmul(pj[:, w16 * 32:(w16 + 1) * 32],
                                             lhsT=XG[:, w, :], rhs=IND[:, w, :],
                                             start=True, stop=True)
                        nc.scalar.activation(XCN[:, bank * 512:(bank + 1) * 512], pj[:], act.Copy)

            # ======== stage C: MLP stack ========
            if stop >= 5:
                with tc.tile_pool(name="hw", bufs=4) as hw, \
                     tc.tile_pool(name="ps", bufs=2, space="PSUM") as ps:
                    ZER = hw.tile([P, 512], dt.float32, tag="zer", bufs=1)
                    nc.vector.memset(ZER[:], 0.0)

                    def layer(src, wt, bcol, kchunks, relu_, tag):
                        fn = act.Relu if relu_ else act.Identity
                        dst = [hw.tile([P, E_LOC], dt.float32, tag=tag,
                                       name=f"{tag}_{bcol}_{m}", bufs=2 if tag != "hwk" else 4)
                               for m in range(2)]
                        for n2 in range(4):
                            nsl2 = slice(n2 * 1024, (n2 + 1) * 1024)
                            for m in range(2):
                                pt = ps.tile([P, 1024], dt.float32, tag="mlp",
                                             name=f"mlp_{bcol}_{n2}_{m}", bufs=3)
                                for h in range(2):
                                    nsl = slice((2 * n2 + h) * 512, (2 * n2 + h + 1) * 512)
                                    for kc in range(kchunks):
                                        if kchunks == 1:
                                            lhs = wt[:, m * 128:(m + 1) * 128]
                                        else:
                                            lhs = wt[:, kc * 256 + m * 128: kc * 256 + (m + 1) * 128]
                                        nc.tensor.matmul(pt[:, h * 512:(h + 1) * 512],
                                                         lhsT=lhs, rhs=src[kc][:, nsl],
                                                         start=(kc == 0), stop=(kc == kchunks - 1))
                                if True:
                                    nc.scalar.activation(dst[m][:, nsl2], pt[:], fn,
                                                         bias=BIA[:, bcol + m: bcol + m + 1])
                                elif relu_:
                                    nc.vector.scalar_tensor_tensor(
                                        out=dst[m][:, nsl2].rearrange("p (a b) -> p a b", a=2),
                                        in0=pt[:].rearrange("p (a b) -> p a b", a=2),
                                        scalar=BIA[:, bcol + m: bcol + m + 1],
                                        in1=ZER[:].unsqueeze(1).to_broadcast([P, 2, 512]),
                                        op0=op.add, op1=op.max)
                                else:
                                    nc.vector.tensor_scalar(
                                        out=dst[m][:, nsl2], in0=pt[:],
                                        scalar1=BIA[:, bcol + m: bcol + m + 1],
                                        scalar2=None, op0=op.add)
                        return dst

                    H1 = layer([XIJT], WX1, 0, 1, True, "h1")
                    HIJ = layer(H1, WX2, 2, 2, False, "hij")
                    C1 = layer([XCN], WC1, 4, 1, True, "hwk")
                    C2 = layer(C1, WC2, 6, 2, True, "hwk")
                    HCN = layer(C2, WC3, 8, 2, False, "hwk")
                    Z = [hw.tile([P, E_LOC], dt.float32, tag="hwk", name=f"z_{m}")
                         for m in range(2)]
                    for m in range(2):
                        nc.vector.scalar_tensor_tensor(out=Z[m][:], in0=HCN[m][:],
                                                       scalar=BIA[:, 13:14], in1=HIJ[m][:],
                                                       op0=op.mult, op1=op.add)
                    G = layer(Z, WL1, 10, 2, True, "hwk")
                    for n in range(8):
                        nsl = slice(n * 512, (n + 1) * 512)
                        pt = ps.tile([1, 512], dt.float32, tag="y", name=f"y_{n}", bufs=1)
                        for kc in range(2):
                            nc.tensor.matmul(pt[:], lhsT=WL2[:, kc:kc + 1], rhs=G[kc][:, nsl],
                                             start=(kc == 0), stop=(kc == 1))
                        nc.scalar.activation(YSB[:, nsl], pt[:], act.Identity,
                                             bias=BIA[:1, 12:13])
                nc.sync.dma_start(y_out[:], YSB[:])

    nc.compile()
    return nc, nm


def _wrap_idx(v):
    """[E_LOC] int array -> [32, E_LOC//16] int16 wrapped + replicated."""
    w = v.reshape(E_LOC // 16, 16).T.astype(np.int16)
    return np.tile(w, (8, 1))


def _pack_w2(w):
    """[256, X] -> [128, 2*X] with k-chunk kc at cols [kc*X, (kc+1)*X)."""
    X = w.shape[1]
    return np.ascontiguousarray(
        w.reshape(2, 128, X).transpose(1, 0, 2).reshape(128, 2 * X)).astype(np.float32)


def kernel(_profile=False, **inputs):
    from concourse.bass_utils import run_bass_kernel_spmd

    x = np.asarray(inputs["x"], np.float32)
    adj = np.asarray(inputs["adj_nbr"], np.int64)
    tar = np.asarray(inputs["tar_ei"], np.int64)
    beta = np.asarray(inputs["beta"], np.float32).reshape(-1)[0]

    if "nc" not in _CACHE:
        _CACHE["nc"], _CACHE["nm"] = _build()
    nc, nm = _CACHE["nc"], _CACHE["nm"]

    adj32 = adj.astype(np.int32)
    adjx = np.ascontiguousarray(np.concatenate([adj32, x.view(np.int32)], axis=1))
    xr = np.ascontiguousarray(x)

    # spread i==j edges so no window gets two of them (64 winners each)
    ti, tj = tar[0], tar[1]
    iijj = np.where(ti == tj)[0]
    perm = np.arange(E_TOT)
    if len(iijj):
        assert len(iijj) * 32 <= E_TOT
        pos = np.arange(len(iijj)) * 32
        mask = np.ones(E_TOT, bool)
        mask[pos] = False
        rest = np.setdiff1d(perm, iijj, assume_unique=True)
        perm = np.empty(E_TOT, np.int64)
        perm[pos] = iijj
        perm[mask] = rest

    # biases packing: [128, 14]
    def b2(b):
        return np.asarray(b, np.float32).reshape(2, 128).T
    BIA = np.zeros((128, 14), np.float32)
    BIA[:, 0:2] = b2(inputs["xij_b1"]); BIA[:, 2:4] = b2(inputs["xij_b2"])
    BIA[:, 4:6] = b2(inputs["xcn_b1"]); BIA[:, 6:8] = b2(inputs["xcn_b2"])
    BIA[:, 8:10] = b2(inputs["xcn_b3"]); BIA[:, 10:12] = b2(inputs["lin_b1"])
    BIA[:, 12] = np.float32(np.asarray(inputs["lin_b2"], np.float32).reshape(-1)[0])
    BIA[:, 13] = beta

    shared = {
        nm["adjx"]: adjx, nm["xrows"]: xr,
        nm["w_xij1"]: np.ascontiguousarray(np.asarray(inputs["xij_w1"], np.float32)),
        nm["w_xij2"]: _pack_w2(np.asarray(inputs["xij_w2"], np.float32)),
        nm["w_cn1"]: np.ascontiguousarray(np.asarray(inputs["xcn_w1"], np.float32)),
        nm["w_cn2"]: _pack_w2(np.asarray(inputs["xcn_w2"], np.float32)),
        nm["w_cn3"]: _pack_w2(np.asarray(inputs["xcn_w3"], np.float32)),
        nm["w_lin1"]: _pack_w2(np.asarray(inputs["lin_w1"], np.float32)),
        nm["w_lin2"]: _pack_w2(np.asarray(inputs["lin_w2"], np.float32)),
        nm["biases"]: BIA,
    }

    t = np.arange(E_LOC)
    e_loc_of_t = (t % 128) * WEDGE + t // 128
    in_maps = []
    for k in range(NCORES):
        ge = perm[k * E_LOC + e_loc_of_t]
        m = dict(shared)
        m[nm["idx_i"]] = _wrap_idx(ti[ge])
        m[nm["idx_j"]] = _wrap_idx(tj[ge])
        in_maps.append(m)

    kw = dict(trace=True, trace_cores=[0]) if _profile else {}
    res = run_bass_kernel_spmd(nc, in_maps, core_ids=list(range(NCORES)), **kw)
    _CACHE["last"] = res
    y = np.empty(E_TOT, np.float32)
    for k in range(NCORES):
        y[perm[k * E_LOC + np.arange(E_LOC)]] = res.results[k][nm["y_out"]].reshape(-1)
    return y.reshape(E_TOT, 1)


# revision 25
# speedup vs baseline: 1.0144x; 1.0118x over previous
"""CNLinkPredictor Trainium2 kernel.

Per-edge common-neighbor link predictor over 8 NeuronCores (data-parallel over
the 32768 target edges, 4096 per core).

Device pipeline per core:
  1. dma_gather adjacency rows adj[i_e], adj[j_e] and feature rows x[i_e], x[j_e]
     (edges laid out 32-per-partition: edge-slot e_loc = p*32 + c).
  2. cm via 64 int16 "rotation" compares on DVE:
     cm[e,a] = OR_b (ni[e,a] == nj[e,b]).
  3. Sparse compaction: per-partition rank (prefix scan) + local_scatter of the
     winning (edge, node) pairs into <=128 slots per partition.
  4. dma_gather of ONLY the winners' x rows; a per-window "selection matmul"
     (gathered rows stationary, 0/1 edge-indicator moving) sums them into
     xcn^T [64, 4096] -- this dedups, masks, and transposes in one PE pass.
  5. Dense f32 MLP stack with features/hidden in partitions, edges streaming in
     the free dimension; y [1, 4096] DMA'd out.
"""

import numpy as np

N, D, E_TOT, F, H = 10000, 64, 32768, 64, 256
NCORES = 8
E_LOC = E_TOT // NCORES      # 4096
P = 128
WEDGE = 32                   # edges per window (= per partition)
NWIN = 128                   # windows per core (= partitions)
KSLOT = 128                  # winner slots per window
NT = WEDGE * D               # 2048 dense (edge,slot) pairs per partition

_CACHE = {}


def _build(stop=5):
    import concourse.bacc as bacc
    import concourse.mybir as mybir
    import concourse.tile as tile
    from concourse.masks import make_identity

    dt = mybir.dt
    op = mybir.AluOpType
    act = mybir.ActivationFunctionType

    nc = bacc.Bacc(None, dynamic_dma_scratch_size=32768, num_swdge_queues=4)
    nm = {}
    with tile.TileContext(nc) as tc:
        with tc.tile_pool(name="dram", bufs=1, space="DRAM") as dram, \
             tc.tile_pool(name="keep", bufs=1) as keep, \
             tc.tile_pool(name="hijp", bufs=1) as hijp:

            # ---------------- DRAM I/O ----------------
            adjx = dram.tile([N, D + F], dt.int32, kind="ExternalInput")
            xrows = dram.tile([N, F], dt.float32, kind="ExternalInput")
            idx_i = dram.tile([128, E_LOC // 16], dt.int16, kind="ExternalInput")
            idx_j = dram.tile([128, E_LOC // 16], dt.int16, kind="ExternalInput")
            w_xij1 = dram.tile([64, 256], dt.float32, kind="ExternalInput")
            w_xij2 = dram.tile([128, 512], dt.float32, kind="ExternalInput")
            w_cn1 = dram.tile([64, 256], dt.float32, kind="ExternalInput")
            w_cn2 = dram.tile([128, 512], dt.float32, kind="ExternalInput")
            w_cn3 = dram.tile([128, 512], dt.float32, kind="ExternalInput")
            w_lin1 = dram.tile([128, 512], dt.float32, kind="ExternalInput")
            w_lin2 = dram.tile([128, 2], dt.float32, kind="ExternalInput")
            biases = dram.tile([128, 14], dt.float32, kind="ExternalInput")
            # bias cols: xij1(2) xij2(2) cn1(2) cn2(2) cn3(2) lin1(2) [lin2, beta]
            y_out = dram.tile([1, E_LOC], dt.float32, kind="ExternalOutput")
            cwd = dram.tile([16, NWIN * KSLOT // 16], dt.int16)

            nm.update(adjx=adjx.name, xrows=xrows.name, idx_i=idx_i.name,
                      idx_j=idx_j.name, w_xij1=w_xij1.name, w_xij2=w_xij2.name,
                      w_cn1=w_cn1.name, w_cn2=w_cn2.name, w_cn3=w_cn3.name,
                      w_lin1=w_lin1.name, w_lin2=w_lin2.name, biases=biases.name,
                      y_out=y_out.name)

            # -------- long-lived tiles --------
            IDENT = keep.tile([P, P], dt.float32)
            make_identity(nc, IDENT[:])
            WX1 = keep.tile([64, 256], dt.float32); nc.sync.dma_start(WX1[:], w_xij1[:])
            WX2 = keep.tile([128, 512], dt.float32); nc.sync.dma_start(WX2[:], w_xij2[:])
            WC1 = keep.tile([64, 256], dt.float32); nc.sync.dma_start(WC1[:], w_cn1[:])
            WC2 = keep.tile([128, 512], dt.float32); nc.sync.dma_start(WC2[:], w_cn2[:])
            WC3 = keep.tile([128, 512], dt.float32); nc.sync.dma_start(WC3[:], w_cn3[:])
            WL1 = keep.tile([128, 512], dt.float32); nc.sync.dma_start(WL1[:], w_lin1[:])
            WL2 = keep.tile([128, 2], dt.float32); nc.sync.dma_start(WL2[:], w_lin2[:])
            BIA = keep.tile([128, 14], dt.float32); nc.sync.dma_start(BIA[:], biases[:])
            XCN = keep.tile([64, E_LOC], dt.float32)
            XIJT = keep.tile([64, E_LOC], dt.float32)
            IDXG = keep.tile([128, NWIN * KSLOT // 16], dt.int16)
            CELT = keep.tile([P, NWIN], dt.float32)
            YSB = keep.tile([1, E_LOC], dt.float32)
            hijp_tiles = {}
            if stop < 5:
                nc.vector.memset(YSB[:], 0.0)
                nc.sync.dma_start(y_out[:], YSB[:])

            def mlp_layer(ps_pool, src, wt, bcol, kchunks, relu_, dst, pbufs=3):
                fn = act.Relu if relu_ else act.Identity
                for n2 in range(4):
                    nsl2 = slice(n2 * 1024, (n2 + 1) * 1024)
                    for m in range(2):
                        pt = ps_pool.tile([P, 1024], dt.float32, tag="mlp",
                                          name=f"mlp_{bcol}_{n2}_{m}", bufs=pbufs)
                        for h in range(2):
                            nsl = slice((2 * n2 + h) * 512, (2 * n2 + h + 1) * 512)
                            for kc in range(kchunks):
                                if kchunks == 1:
                                    lhs = wt[:, m * 128:(m + 1) * 128]
                                else:
                                    lhs = wt[:, kc * 256 + m * 128: kc * 256 + (m + 1) * 128]
                                nc.tensor.matmul(pt[:, h * 512:(h + 1) * 512],
                                                 lhsT=lhs, rhs=src[kc][:, nsl],
                                                 start=(kc == 0), stop=(kc == kchunks - 1))
                        nc.scalar.activation(dst[m][:, nsl2], pt[:], fn,
                                             bias=BIA[:, bcol + m: bcol + m + 1])
                return dst

            # ======== stage A: gathers, xij^T, cm, compaction ========
            with tc.tile_pool(name="sa", bufs=1) as sa, \
                 tc.tile_pool(name="wk", bufs=2) as wk, \
                 tc.tile_pool(name="ps", bufs=2, space="PSUM") as ps:
                IDXI = sa.tile([128, E_LOC // 16], dt.int16)
                nc.sync.dma_start(IDXI[:], idx_i[:])
                IDXJ = sa.tile([128, E_LOC // 16], dt.int16)
                nc.sync.dma_start(IDXJ[:], idx_j[:])

                AXI = sa.tile([P, WEDGE, D + F], dt.int32)
                for ch in range(4):
                    nc.gpsimd.dma_gather(AXI[:, ch * 8:(ch + 1) * 8, :], adjx[:],
                                         IDXI[:, ch * 64:(ch + 1) * 64],
                                         num_idxs=1024, num_idxs_reg=1024,
                                         elem_size=D + F, queue_num=ch)
                AXJ = sa.tile([P, WEDGE, D + F], dt.int32)
                for ch in range(4):
                    nc.gpsimd.dma_gather(AXJ[:, ch * 8:(ch + 1) * 8, :], adjx[:],
                                         IDXJ[:, ch * 64:(ch + 1) * 64],
                                         num_idxs=1024, num_idxs_reg=1024,
                                         elem_size=D + F, queue_num=ch)
                NI32 = AXI[:, :, 0:D]
                NJ32 = AXJ[:, :, 0:D]
                XI = AXI[:, :, D:D + F].bitcast(dt.float32)
                XJ = AXJ[:, :, D:D + F].bitcast(dt.float32)

                # xij^T via PE transposes (early, frees XI/XJ)
                XIJ = sa.tile([P, WEDGE, F], dt.float32)
                nc.vector.tensor_tensor(out=XIJ[:], in0=XI, in1=XJ, op=op.mult)
                xijt_v = XIJT[:].rearrange("f (p bk q) -> f bk q p", bk=8, q=4)
                for bank in range(8):
                    pt = ps.tile([64, 4, 128], dt.float32, tag="sel", name=f"ptr{bank}")
                    for q in range(4):
                        c = bank * 4 + q
                        nc.tensor.matmul(pt[:, q, :], lhsT=XIJ[:, c, :], rhs=IDENT[:],
                                         start=True, stop=True)
                    nc.scalar.activation(xijt_v[:, bank], pt[:], act.Copy)

                if stop >= 5:
                    H1 = [hijp.tile([P, E_LOC], dt.float32, name=f"h1_{m}")
                          for m in range(2)]
                    mlp_layer(ps, [XIJT], WX1, 0, 1, True, H1, pbufs=2)

                if stop >= 2:
                    # int16 neighbor ids
                    NI16 = sa.tile([P, WEDGE, D], dt.int16)
                    nc.vector.tensor_copy(NI16[:], NI32)
                    NIP116 = sa.tile([P, WEDGE, D], dt.int16)
                    nc.vector.tensor_scalar(out=NIP116[:], in0=NI16[:],
                                            scalar1=1, scalar2=None, op0=op.add)
                    NJd = sa.tile([P, WEDGE, 2 * D], dt.int16)
                    nc.vector.tensor_copy(NJd[:, :, 0:D], NJ32)
                    nc.vector.tensor_copy(NJd[:, :, D:2 * D], NJd[:, :, 0:D])
                    # 4B-aligned odd-shift copy so every rotation slice hits
                    # the DVE 2x perf mode (odd element offsets fall to 1x)
                    NJe = sa.tile([P, WEDGE, 2 * D], dt.int16)
                    nc.vector.tensor_copy(NJe[:, :, 0:2 * D - 1], NJd[:, :, 1:2 * D])

                    # cm rotation loop
                    ACC = sa.tile([P, WEDGE, D], dt.int16)
                    nc.vector.memset(ACC[:], 0)
                    for r in range(D):
                        EQ = wk.tile([P, WEDGE, D], dt.int16, tag="eq", name=f"eq{r}", bufs=1)
                        src = NJd[:, :, r:r + D] if r % 2 == 0 \
                            else NJe[:, :, r - 1:r - 1 + D]
                        nc.vector.tensor_tensor(out=EQ[:], in0=NI16[:],
                                                in1=src, op=op.is_equal)
                        nc.vector.tensor_tensor(out=ACC[:], in0=ACC[:], in1=EQ[:],
                                                op=op.max)

                if stop >= 3:
                    # compaction
                    CMF = sa.tile([P, NT], dt.float32)
                    nc.vector.tensor_copy(CMF[:], ACC[:].rearrange("p a b -> p (a b)"))
                    ONES = sa.tile([P, NT], dt.bfloat16)
                    nc.vector.memset(ONES[:], 1.0)
                    RANK = sa.tile([P, NT], dt.float32)
                    nc.vector.tensor_tensor_scan(RANK[:], ONES[:], CMF[:], 0.0,
                                                 op.mult, op.add)
                    nc.vector.tensor_tensor(out=RANK[:], in0=RANK[:], in1=CMF[:], op=op.mult)
                    nc.vector.tensor_scalar(out=RANK[:], in0=RANK[:], scalar1=-1.0,
                                            scalar2=None, op0=op.add)
                    nc.vector.tensor_scalar(out=RANK[:], in0=RANK[:],
                                            scalar1=float(KSLOT - 1), scalar2=None, op0=op.min)
                    SLOT = sa.tile([P, NT], dt.int16)
                    nc.vector.tensor_copy(SLOT[:], RANK[:])
                    CEL16 = wk.tile([P, NT], dt.int16, tag="eq", bufs=1)
                    nc.gpsimd.iota(CEL16[:], pattern=[[1, WEDGE], [0, D]], base=1,
                                   channel_multiplier=0)
                    CW = sa.tile([P, KSLOT], dt.int16)
                    nc.gpsimd.local_scatter(CW[:], NIP116[:].rearrange("p a b -> p (a b)"),
                                            SLOT[:], channels=P,
                                            num_elems=KSLOT, num_idxs=NT)
                    CEL = sa.tile([P, KSLOT], dt.int16)
                    nc.gpsimd.local_scatter(CEL[:], CEL16[:], SLOT[:], channels=P,
                                            num_elems=KSLOT, num_idxs=NT)
                    CWm1 = sa.tile([P, KSLOT], dt.int16)
                    nc.vector.tensor_scalar(out=CWm1[:], in0=CW[:], scalar1=-1,
                                            scalar2=None, op0=op.add)
                    nc.vector.tensor_scalar(out=CWm1[:], in0=CWm1[:], scalar1=0,
                                            scalar2=None, op0=op.max)
                    # write CWm1 to DRAM already in wrapped [16, 1024] layout:
                    # dst flat(p16, w*8+kh) <- CWm1[w, k=kh*16+p16]
                    cwd_wv = cwd[:].rearrange("p (w kh) -> w kh p", kh=8)
                    nc.sync.dma_start(cwd_wv, CWm1[:].rearrange("w (kh p) -> w kh p", p=16))
                    for g in range(8):
                        nc.sync.dma_start(IDXG[16 * g:16 * (g + 1), :], cwd[:])

                    CELF = sa.tile([P, KSLOT], dt.float32)
                    nc.vector.tensor_copy(CELF[:], CEL[:])
                    CELT_ps = ps.tile([P, P], dt.float32, tag="tr", bufs=1)
                    nc.tensor.transpose(CELT_ps[:], CELF[:], IDENT[:])
                    nc.vector.tensor_copy(CELT[:], CELT_ps[:])

            # ======== stage B: winner gather + selection matmuls ========
            if stop >= 4:
                with tc.tile_pool(name="sbp", bufs=1) as sbp, \
                     tc.tile_pool(name="ps", bufs=2, space="PSUM") as ps:
                    XG = sbp.tile([P, NWIN, F], dt.float32)
                    for ch in range(16):
                        nc.gpsimd.dma_gather(XG[:, ch * 8:(ch + 1) * 8, :], xrows[:],
                                             IDXG[:, ch * 64:(ch + 1) * 64],
                                             num_idxs=1024, num_idxs_reg=1024,
                                             elem_size=F, queue_num=ch % 4)
                    INDN = sbp.tile([P, WEDGE], dt.int32)
                    nc.gpsimd.iota(INDN[:], pattern=[[1, WEDGE]], base=1, channel_multiplier=0)
                    INDNF = sbp.tile([P, WEDGE], dt.float32)
                    nc.vector.tensor_copy(INDNF[:], INDN[:])
                    IND = sbp.tile([P, NWIN, WEDGE], dt.float32)
                    nc.vector.tensor_tensor(out=IND[:],
                                            in0=CELT[:].to_broadcast([P, NWIN, WEDGE]),
                                            in1=INDNF[:].unsqueeze(1).to_broadcast([P, NWIN, WEDGE]),
                                            op=op.is_equal)
                    for bank in range(8):
                        pj = ps.tile([64, 512], dt.float32, tag="sel", name=f"pj{bank}")
                        for w16 in range(16):
                            w = bank * 16 + w16
                            nc.tensor.matmul(pj[:, w16 * 32:(w16 + 1) * 32],
                                             lhsT=XG[:, w, :], rhs=IND[:, w, :],
                                             start=True, stop=True)
                        nc.scalar.activation(XCN[:, bank * 512:(bank + 1) * 512], pj[:], act.Copy)

            # ======== stage C: MLP stack ========
            if stop >= 5:
                with tc.tile_pool(name="hw", bufs=4) as hw, \
                     tc.tile_pool(name="ps", bufs=2, space="PSUM") as ps:
                    def ltile(tag, nmx, b=4):
                        return [hw.tile([P, E_LOC], dt.float32, tag=tag,
                                        name=f"{nmx}_{m}", bufs=b) for m in range(2)]
                    HIJ = ltile("hij", "hij", b=2)
                    mlp_layer(ps, H1, WX2, 2, 2, False, HIJ)
                    C1 = ltile("hwk", "c1")
                    mlp_layer(ps, [XCN], WC1, 4, 1, True, C1)
                    C2 = ltile("hwk", "c2")
                    mlp_layer(ps, C1, WC2, 6, 2, True, C2)
                    HCN = ltile("hwk", "hcn")
                    mlp_layer(ps, C2, WC3, 8, 2, False, HCN)
                    Z = ltile("hwk", "z")
                    for m in range(2):
                        nc.vector.scalar_tensor_tensor(out=Z[m][:], in0=HCN[m][:],
                                                       scalar=BIA[:, 13:14], in1=HIJ[m][:],
                                                       op0=op.mult, op1=op.add)
                    G = ltile("hwk", "g")
                    mlp_layer(ps, Z, WL1, 10, 2, True, G)
                    for n in range(8):
                        nsl = slice(n * 512, (n + 1) * 512)
                        pt = ps.tile([1, 512], dt.float32, tag="y", name=f"y_{n}", bufs=1)
                        for kc in range(2):
                            nc.tensor.matmul(pt[:], lhsT=WL2[:, kc:kc + 1], rhs=G[kc][:, nsl],
                                             start=(kc == 0), stop=(kc == 1))
                        nc.scalar.activation(YSB[:, nsl], pt[:], act.Identity,
                                             bias=BIA[:1, 12:13])
                nc.sync.dma_start(y_out[:], YSB[:])

    nc.compile()
    return nc, nm


def _wrap_idx(v):
    """[E_LOC] int array -> [32, E_LOC//16] int16 wrapped + replicated."""
    w = v.reshape(E_LOC // 16, 16).T.astype(np.int16)
    return np.tile(w, (8, 1))


def _pack_w2(w):
    """[256, X] -> [128, 2*X] with k-chunk kc at cols [kc*X, (kc+1)*X)."""
    X = w.shape[1]
    return np.ascontiguousarray(
        w.reshape(2, 128, X).transpose(1, 0, 2).reshape(128, 2 * X)).astype(np.float32)


def kernel(_profile=False, **inputs):
    from concourse.bass_utils import run_bass_kernel_spmd

    x = np.asarray(inputs["x"], np.float32)
    adj = np.asarray(inputs["adj_nbr"], np.int64)
    tar = np.asarray(inputs["tar_ei"], np.int64)
    beta = np.asarray(inputs["beta"], np.float32).reshape(-1)[0]

    if "nc" not in _CACHE:
        _CACHE["nc"], _CACHE["nm"] = _build()
    nc, nm = _CACHE["nc"], _CACHE["nm"]

    adj32 = adj.astype(np.int32)
    adjx = np.ascontiguousarray(np.concatenate([adj32, x.view(np.int32)], axis=1))
    xr = np.ascontiguousarray(x)

    # spread i==j edges so no window gets two of them (64 winners each)
    ti, tj = tar[0], tar[1]
    iijj = np.where(ti == tj)[0]
    perm = np.arange(E_TOT)
    if len(iijj):
        assert len(iijj) * 32 <= E_TOT
        pos = np.arange(len(iijj)) * 32
        mask = np.ones(E_TOT, bool)
        mask[pos] = False
        rest = np.setdiff1d(perm, iijj, assume_unique=True)
        perm = np.empty(E_TOT, np.int64)
        perm[pos] = iijj
        perm[mask] = rest

    # biases packing: [128, 14]
    def b2(b):
        return np.asarray(b, np.float32).reshape(2, 128).T
    BIA = np.zeros((128, 14), np.float32)
    BIA[:, 0:2] = b2(inputs["xij_b1"]); BIA[:, 2:4] = b2(inputs["xij_b2"])
    BIA[:, 4:6] = b2(inputs["xcn_b1"]); BIA[:, 6:8] = b2(inputs["xcn_b2"])
    BIA[:, 8:10] = b2(inputs["xcn_b3"]); BIA[:, 10:12] = b2(inputs["lin_b1"])
    BIA[:, 12] = np.float32(np.asarray(inputs["lin_b2"], np.float32).reshape(-1)[0])
    BIA[:, 13] = beta

    shared = {
        nm["adjx"]: adjx, nm["xrows"]: xr,
        nm["w_xij1"]: np.ascontiguousarray(np.asarray(inputs["xij_w1"], np.float32)),
        nm["w_xij2"]: _pack_w2(np.asarray(inputs["xij_w2"], np.float32)),
        nm["w_cn1"]: np.ascontiguousarray(np.asarray(inputs["xcn_w1"], np.float32)),
        nm["w_cn2"]: _pack_w2(np.asarray(inputs["xcn_w2"], np.float32)),
        nm["w_cn3"]: _pack_w2(np.asarray(inputs["xcn_w3"], np.float32)),
        nm["w_lin1"]: _pack_w2(np.asarray(inputs["lin_w1"], np.float32)),
        nm["w_lin2"]: _pack_w2(np.asarray(inputs["lin_w2"], np.float32)),
        nm["biases"]: BIA,
    }

    t = np.arange(E_LOC)
    e_loc_of_t = (t % 128) * WEDGE + t // 128
    in_maps = []
    for k in range(NCORES):
        ge = perm[k * E_LOC + e_loc_of_t]
        m = dict(shared)
        m[nm["idx_i"]] = _wrap_idx(ti[ge])
        m[nm["idx_j"]] = _wrap_idx(tj[ge])
        in_maps.append(m)

    kw = dict(trace=True, trace_cores=[0]) if _profile else {}
    res = run_bass_kernel_spmd(nc, in_maps, core_ids=list(range(NCORES)), **kw)
    _CACHE["last"] = res
    y = np.empty(E_TOT, np.float32)
    for k in range(NCORES):
        y[perm[k * E_LOC + np.arange(E_LOC)]] = res.results[k][nm["y_out"]].reshape(-1)
    return y.reshape(E_TOT, 1)


# revision 26
# speedup vs baseline: 1.0305x; 1.0159x over previous
"""CNLinkPredictor Trainium2 kernel.

Per-edge common-neighbor link predictor over 8 NeuronCores (data-parallel over
the 32768 target edges, 4096 per core).

Device pipeline per core:
  1. dma_gather adjacency rows adj[i_e], adj[j_e] and feature rows x[i_e], x[j_e]
     (edges laid out 32-per-partition: edge-slot e_loc = p*32 + c).
  2. cm via 64 int16 "rotation" compares on DVE:
     cm[e,a] = OR_b (ni[e,a] == nj[e,b]).
  3. Sparse compaction: per-partition rank (prefix scan) + local_scatter of the
     winning (edge, node) pairs into <=128 slots per partition.
  4. dma_gather of ONLY the winners' x rows; a per-window "selection matmul"
     (gathered rows stationary, 0/1 edge-indicator moving) sums them into
     xcn^T [64, 4096] -- this dedups, masks, and transposes in one PE pass.
  5. Dense f32 MLP stack with features/hidden in partitions, edges streaming in
     the free dimension; y [1, 4096] DMA'd out.
"""

import numpy as np

N, D, E_TOT, F, H = 10000, 64, 32768, 64, 256
NCORES = 8
E_LOC = E_TOT // NCORES      # 4096
P = 128
WEDGE = 32                   # edges per window (= per partition)
NWIN = 128                   # windows per core (= partitions)
KSLOT = 128                  # winner slots per window
NT = WEDGE * D               # 2048 dense (edge,slot) pairs per partition

_CACHE = {}


def _build(stop=5):
    import concourse.bacc as bacc
    import concourse.mybir as mybir
    import concourse.tile as tile
    from concourse.masks import make_identity

    dt = mybir.dt
    op = mybir.AluOpType
    act = mybir.ActivationFunctionType

    nc = bacc.Bacc(None, dynamic_dma_scratch_size=32768, num_swdge_queues=4)
    nm = {}
    with tile.TileContext(nc) as tc:
        with tc.tile_pool(name="dram", bufs=1, space="DRAM") as dram, \
             tc.tile_pool(name="keep", bufs=1) as keep, \
             tc.tile_pool(name="hijp", bufs=1) as hijp:

            # ---------------- DRAM I/O ----------------
            adjx = dram.tile([N, D + F], dt.int32, kind="ExternalInput")
            xrows = dram.tile([N, F], dt.float32, kind="ExternalInput")
            idx_i = dram.tile([128, E_LOC // 16], dt.int16, kind="ExternalInput")
            idx_j = dram.tile([128, E_LOC // 16], dt.int16, kind="ExternalInput")
            w_xij1 = dram.tile([64, 256], dt.float32, kind="ExternalInput")
            w_xij2 = dram.tile([128, 512], dt.float32, kind="ExternalInput")
            w_cn1 = dram.tile([64, 256], dt.float32, kind="ExternalInput")
            w_cn2 = dram.tile([128, 512], dt.float32, kind="ExternalInput")
            w_cn3 = dram.tile([128, 512], dt.float32, kind="ExternalInput")
            w_lin1 = dram.tile([128, 512], dt.float32, kind="ExternalInput")
            w_lin2 = dram.tile([128, 2], dt.float32, kind="ExternalInput")
            biases = dram.tile([128, 14], dt.float32, kind="ExternalInput")
            # bias cols: xij1(2) xij2(2) cn1(2) cn2(2) cn3(2) lin1(2) [lin2, beta]
            y_out = dram.tile([1, E_LOC], dt.float32, kind="ExternalOutput")
            cwd = dram.tile([16, NWIN * KSLOT // 16], dt.int16)

            nm.update(adjx=adjx.name, xrows=xrows.name, idx_i=idx_i.name,
                      idx_j=idx_j.name, w_xij1=w_xij1.name, w_xij2=w_xij2.name,
                      w_cn1=w_cn1.name, w_cn2=w_cn2.name, w_cn3=w_cn3.name,
                      w_lin1=w_lin1.name, w_lin2=w_lin2.name, biases=biases.name,
                      y_out=y_out.name)

            # -------- long-lived tiles --------
            IDENT = keep.tile([P, P], dt.float32)
            make_identity(nc, IDENT[:])
            WX1 = keep.tile([64, 256], dt.float32); nc.sync.dma_start(WX1[:], w_xij1[:])
            WX2 = keep.tile([128, 512], dt.float32); nc.sync.dma_start(WX2[:], w_xij2[:])
            WC1 = keep.tile([64, 256], dt.float32); nc.sync.dma_start(WC1[:], w_cn1[:])
            WC2 = keep.tile([128, 512], dt.float32); nc.sync.dma_start(WC2[:], w_cn2[:])
            WC3 = keep.tile([128, 512], dt.float32); nc.sync.dma_start(WC3[:], w_cn3[:])
            WL1 = keep.tile([128, 512], dt.float32); nc.sync.dma_start(WL1[:], w_lin1[:])
            WL2 = keep.tile([128, 2], dt.float32); nc.sync.dma_start(WL2[:], w_lin2[:])
            BIA = keep.tile([128, 14], dt.float32); nc.sync.dma_start(BIA[:], biases[:])
            XCN = keep.tile([64, E_LOC], dt.float32)
            XIJT = keep.tile([64, E_LOC], dt.float32)
            IDXG = keep.tile([128, NWIN * KSLOT // 16], dt.int16)
            CELT = keep.tile([P, NWIN], dt.float32)
            YSB = keep.tile([1, E_LOC], dt.float32)
            hijp_tiles = {}
            if stop < 5:
                nc.vector.memset(YSB[:], 0.0)
                nc.sync.dma_start(y_out[:], YSB[:])

            def mlp_layer(ps_pool, src, wt, bcol, kchunks, relu_, dst, pbufs=3):
                fn = act.Relu if relu_ else act.Identity
                for n2 in range(4):
                    nsl2 = slice(n2 * 1024, (n2 + 1) * 1024)
                    for m in range(2):
                        pt = ps_pool.tile([P, 1024], dt.float32, tag="mlp",
                                          name=f"mlp_{bcol}_{n2}_{m}", bufs=pbufs)
                        for h in range(2):
                            nsl = slice((2 * n2 + h) * 512, (2 * n2 + h + 1) * 512)
                            for kc in range(kchunks):
                                if kchunks == 1:
                                    lhs = wt[:, m * 128:(m + 1) * 128]
                                else:
                                    lhs = wt[:, kc * 256 + m * 128: kc * 256 + (m + 1) * 128]
                                nc.tensor.matmul(pt[:, h * 512:(h + 1) * 512],
                                                 lhsT=lhs, rhs=src[kc][:, nsl],
                                                 start=(kc == 0), stop=(kc == kchunks - 1))
                        nc.scalar.activation(dst[m][:, nsl2], pt[:], fn,
                                             bias=BIA[:, bcol + m: bcol + m + 1])
                return dst

            # ======== stage A: gathers, xij^T, cm, compaction ========
            with tc.tile_pool(name="sa", bufs=1) as sa, \
                 tc.tile_pool(name="wk", bufs=2) as wk, \
                 tc.tile_pool(name="ps", bufs=2, space="PSUM") as ps:
                IDXI = sa.tile([128, E_LOC // 16], dt.int16)
                nc.sync.dma_start(IDXI[:], idx_i[:])
                IDXJ = sa.tile([128, E_LOC // 16], dt.int16)
                nc.sync.dma_start(IDXJ[:], idx_j[:])

                AXI = sa.tile([P, WEDGE, D + F], dt.int32)
                for ch in range(4):
                    nc.gpsimd.dma_gather(AXI[:, ch * 8:(ch + 1) * 8, :], adjx[:],
                                         IDXI[:, ch * 64:(ch + 1) * 64],
                                         num_idxs=1024, num_idxs_reg=1024,
                                         elem_size=D + F, queue_num=ch)
                AXJ = sa.tile([P, WEDGE, D + F], dt.int32)
                for ch in range(4):
                    nc.gpsimd.dma_gather(AXJ[:, ch * 8:(ch + 1) * 8, :], adjx[:],
                                         IDXJ[:, ch * 64:(ch + 1) * 64],
                                         num_idxs=1024, num_idxs_reg=1024,
                                         elem_size=D + F, queue_num=ch)
                NI32 = AXI[:, :, 0:D]
                NJ32 = AXJ[:, :, 0:D]
                XI = AXI[:, :, D:D + F].bitcast(dt.float32)
                XJ = AXJ[:, :, D:D + F].bitcast(dt.float32)

                # xij^T via PE transposes (early, frees XI/XJ)
                XIJ = sa.tile([P, WEDGE, F], dt.float32)
                nc.vector.tensor_tensor(out=XIJ[:], in0=XI, in1=XJ, op=op.mult)
                xijt_v = XIJT[:].rearrange("f (p bk q) -> f bk q p", bk=8, q=4)
                for bank in range(8):
                    pt = ps.tile([64, 4, 128], dt.float32, tag="sel", name=f"ptr{bank}")
                    for q in range(4):
                        c = bank * 4 + q
                        nc.tensor.matmul(pt[:, q, :], lhsT=XIJ[:, c, :], rhs=IDENT[:],
                                         start=True, stop=True)
                    nc.scalar.activation(xijt_v[:, bank], pt[:], act.Copy)

                if stop >= 5:
                    H1 = [hijp.tile([P, E_LOC], dt.float32, name=f"h1_{m}")
                          for m in range(2)]
                    mlp_layer(ps, [XIJT], WX1, 0, 1, True, H1, pbufs=2)

                if stop >= 2:
                    # int16 neighbor ids
                    NI16 = sa.tile([P, WEDGE, D], dt.int16)
                    nc.vector.tensor_copy(NI16[:], NI32)
                    NIP116 = sa.tile([P, WEDGE, D], dt.int16)
                    nc.vector.tensor_scalar(out=NIP116[:], in0=NI16[:],
                                            scalar1=1, scalar2=None, op0=op.add)
                    NJd = sa.tile([P, WEDGE, 2 * D], dt.int16)
                    nc.vector.tensor_copy(NJd[:, :, 0:D], NJ32)
                    nc.vector.tensor_copy(NJd[:, :, D:2 * D], NJd[:, :, 0:D])
                    # 4B-aligned odd-shift copy so every rotation slice hits
                    # the DVE 2x perf mode (odd element offsets fall to 1x)
                    NJe = sa.tile([P, WEDGE, 2 * D], dt.int16)
                    nc.vector.tensor_copy(NJe[:, :, 0:2 * D - 1], NJd[:, :, 1:2 * D])

                    # cm rotation loop
                    ACC = sa.tile([P, WEDGE, D], dt.int16)
                    nc.vector.memset(ACC[:], 0)
                    for r in range(D):
                        EQ = wk.tile([P, WEDGE, D], dt.int16, tag="eq", name=f"eq{r}", bufs=1)
                        src = NJd[:, :, r:r + D] if r % 2 == 0 \
                            else NJe[:, :, r - 1:r - 1 + D]
                        nc.vector.tensor_tensor(out=EQ[:], in0=NI16[:],
                                                in1=src, op=op.is_equal)
                        nc.vector.tensor_tensor(out=ACC[:], in0=ACC[:], in1=EQ[:],
                                                op=op.max)

                if stop >= 3:
                    # compaction
                    CMF = sa.tile([P, NT], dt.float32)
                    nc.vector.tensor_copy(CMF[:], ACC[:].rearrange("p a b -> p (a b)"))
                    ONES = sa.tile([P, NT], dt.bfloat16)
                    nc.vector.memset(ONES[:], 1.0)
                    RANK = sa.tile([P, NT], dt.float32)
                    nc.vector.tensor_tensor_scan(RANK[:], ONES[:], CMF[:], 0.0,
                                                 op.mult, op.add)
                    nc.vector.tensor_tensor(out=RANK[:], in0=RANK[:], in1=CMF[:], op=op.mult)
                    nc.vector.tensor_scalar(out=RANK[:], in0=RANK[:], scalar1=-1.0,
                                            scalar2=None, op0=op.add)
                    nc.vector.tensor_scalar(out=RANK[:], in0=RANK[:],
                                            scalar1=float(KSLOT - 1), scalar2=None, op0=op.min)
                    SLOT = sa.tile([P, NT], dt.int16)
                    nc.vector.tensor_copy(SLOT[:], RANK[:])
                    CEL16 = wk.tile([P, NT], dt.int16, tag="eq", bufs=1)
                    nc.gpsimd.iota(CEL16[:], pattern=[[1, WEDGE], [0, D]], base=1,
                                   channel_multiplier=0)
                    CW = sa.tile([P, KSLOT], dt.int16)
                    nc.gpsimd.local_scatter(CW[:], NIP116[:].rearrange("p a b -> p (a b)"),
                                            SLOT[:], channels=P,
                                            num_elems=KSLOT, num_idxs=NT)
                    CEL = sa.tile([P, KSLOT], dt.int16)
                    nc.gpsimd.local_scatter(CEL[:], CEL16[:], SLOT[:], channels=P,
                                            num_elems=KSLOT, num_idxs=NT)
                    CWm1 = sa.tile([P, KSLOT], dt.int16)
                    nc.vector.tensor_scalar(out=CWm1[:], in0=CW[:], scalar1=-1,
                                            scalar2=None, op0=op.add)
                    nc.vector.tensor_scalar(out=CWm1[:], in0=CWm1[:], scalar1=0,
                                            scalar2=None, op0=op.max)
                    # write CWm1 to DRAM already in wrapped [16, 1024] layout:
                    # dst flat(p16, w*8+kh) <- CWm1[w, k=kh*16+p16]
                    cwd_wv = cwd[:].rearrange("p (w kh) -> w kh p", kh=8)
                    nc.sync.dma_start(cwd_wv, CWm1[:].rearrange("w (kh p) -> w kh p", p=16))
                    for g in range(8):
                        nc.sync.dma_start(IDXG[16 * g:16 * (g + 1), :], cwd[:])

                    CELF = sa.tile([P, KSLOT], dt.float32)
                    nc.vector.tensor_copy(CELF[:], CEL[:])
                    CELT_ps = ps.tile([P, P], dt.float32, tag="tr", bufs=1)
                    nc.tensor.transpose(CELT_ps[:], CELF[:], IDENT[:])
                    nc.vector.tensor_copy(CELT[:], CELT_ps[:])

            # ======== stage B: winner gather + selection matmuls ========
            if stop >= 4:
                with tc.tile_pool(name="sbp", bufs=1) as sbp, \
                     tc.tile_pool(name="ps", bufs=2, space="PSUM") as ps:
                    XG = sbp.tile([P, NWIN, F], dt.float32)
                    for ch in range(16):
                        nc.gpsimd.dma_gather(XG[:, ch * 8:(ch + 1) * 8, :], xrows[:],
                                             IDXG[:, ch * 64:(ch + 1) * 64],
                                             num_idxs=1024, num_idxs_reg=1024,
                                             elem_size=F, queue_num=ch % 4)
                    INDN = sbp.tile([P, WEDGE], dt.int32)
                    nc.gpsimd.iota(INDN[:], pattern=[[1, WEDGE]], base=1, channel_multiplier=0)
                    INDNF = sbp.tile([P, WEDGE], dt.float32)
                    nc.vector.tensor_copy(INDNF[:], INDN[:])
                    IND = sbp.tile([P, NWIN, WEDGE], dt.float32)
                    nc.vector.tensor_tensor(out=IND[:],
                                            in0=CELT[:].to_broadcast([P, NWIN, WEDGE]),
                                            in1=INDNF[:].unsqueeze(1).to_broadcast([P, NWIN, WEDGE]),
                                            op=op.is_equal)
                    for bank in range(8):
                        pj = ps.tile([64, 512], dt.float32, tag="sel", name=f"pj{bank}")
                        for w16 in range(16):
                            w = bank * 16 + w16
                            nc.tensor.matmul(pj[:, w16 * 32:(w16 + 1) * 32],
                                             lhsT=XG[:, w, :], rhs=IND[:, w, :],
                                             start=True, stop=True)
                        nc.scalar.activation(XCN[:, bank * 512:(bank + 1) * 512], pj[:], act.Copy)

            # ======== stage C: MLP stack ========
            if stop >= 5:
                with tc.tile_pool(name="hw", bufs=4) as hw, \
                     tc.tile_pool(name="ps", bufs=2, space="PSUM") as ps:
                    def ltile(tag, nmx, b=4):
                        return [hw.tile([P, E_LOC], dt.float32, tag=tag,
                                        name=f"{nmx}_{m}", bufs=b) for m in range(2)]
                    HIJ = ltile("hij", "hij", b=2)
                    mlp_layer(ps, H1, WX2, 2, 2, False, HIJ, pbufs=4)
                    C1 = ltile("hwk", "c1")
                    mlp_layer(ps, [XCN], WC1, 4, 1, True, C1, pbufs=4)
                    C2 = ltile("hwk", "c2")
                    mlp_layer(ps, C1, WC2, 6, 2, True, C2, pbufs=4)
                    HCN = ltile("hwk", "hcn")
                    mlp_layer(ps, C2, WC3, 8, 2, False, HCN, pbufs=4)
                    Z = ltile("hwk", "z")
                    for m in range(2):
                        nc.vector.scalar_tensor_tensor(out=Z[m][:], in0=HCN[m][:],
                                                       scalar=BIA[:, 13:14], in1=HIJ[m][:],
                                                       op0=op.mult, op1=op.add)
                    G = ltile("hwk", "g")
                    mlp_layer(ps, Z, WL1, 10, 2, True, G, pbufs=4)
                    for n in range(8):
                        nsl = slice(n * 512, (n + 1) * 512)
                        pt = ps.tile([1, 512], dt.float32, tag="mlp", name=f"y_{n}", bufs=4)
                        for kc in range(2):
                            nc.tensor.matmul(pt[:], lhsT=WL2[:, kc:kc + 1], rhs=G[kc][:, nsl],
                                             start=(kc == 0), stop=(kc == 1))
                        nc.scalar.activation(YSB[:, nsl], pt[:], act.Identity,
                                             bias=BIA[:1, 12:13])
                nc.sync.dma_start(y_out[:], YSB[:])

    nc.compile()
    return nc, nm


def _wrap_idx(v):
    """[E_LOC] int array -> [32, E_LOC//16] int16 wrapped + replicated."""
    w = v.reshape(E_LOC // 16, 16).T.astype(np.int16)
    return np.tile(w, (8, 1))


def _pack_w2(w):
    """[256, X] -> [128, 2*X] with k-chunk kc at cols [kc*X, (kc+1)*X)."""
    X = w.shape[1]
    return np.ascontiguousarray(
        w.reshape(2, 128, X).transpose(1, 0, 2).reshape(128, 2 * X)).astype(np.float32)


def kernel(_profile=False, **inputs):
    from concourse.bass_utils import run_bass_kernel_spmd

    x = np.asarray(inputs["x"], np.float32)
    adj = np.asarray(inputs["adj_nbr"], np.int64)
    tar = np.asarray(inputs["tar_ei"], np.int64)
    beta = np.asarray(inputs["beta"], np.float32).reshape(-1)[0]

    if "nc" not in _CACHE:
        _CACHE["nc"], _CACHE["nm"] = _build()
    nc, nm = _CACHE["nc"], _CACHE["nm"]

    adj32 = adj.astype(np.int32)
    adjx = np.ascontiguousarray(np.concatenate([adj32, x.view(np.int32)], axis=1))
    xr = np.ascontiguousarray(x)

    # spread i==j edges so no window gets two of them (64 winners each)
    ti, tj = tar[0], tar[1]
    iijj = np.where(ti == tj)[0]
    perm = np.arange(E_TOT)
    if len(iijj):
        assert len(iijj) * 32 <= E_TOT
        pos = np.arange(len(iijj)) * 32
        mask = np.ones(E_TOT, bool)
        mask[pos] = False
        rest = np.setdiff1d(perm, iijj, assume_unique=True)
        perm = np.empty(E_TOT, np.int64)
        perm[pos] = iijj
        perm[mask] = rest

    # biases packing: [128, 14]
    def b2(b):
        return np.asarray(b, np.float32).reshape(2, 128).T
    BIA = np.zeros((128, 14), np.float32)
    BIA[:, 0:2] = b2(inputs["xij_b1"]); BIA[:, 2:4] = b2(inputs["xij_b2"])
    BIA[:, 4:6] = b2(inputs["xcn_b1"]); BIA[:, 6:8] = b2(inputs["xcn_b2"])
    BIA[:, 8:10] = b2(inputs["xcn_b3"]); BIA[:, 10:12] = b2(inputs["lin_b1"])
    BIA[:, 12] = np.float32(np.asarray(inputs["lin_b2"], np.float32).reshape(-1)[0])
    BIA[:, 13] = beta

    shared = {
        nm["adjx"]: adjx, nm["xrows"]: xr,
        nm["w_xij1"]: np.ascontiguousarray(np.asarray(inputs["xij_w1"], np.float32)),
        nm["w_xij2"]: _pack_w2(np.asarray(inputs["xij_w2"], np.float32)),
        nm["w_cn1"]: np.ascontiguousarray(np.asarray(inputs["xcn_w1"], np.float32)),
        nm["w_cn2"]: _pack_w2(np.asarray(inputs["xcn_w2"], np.float32)),
        nm["w_cn3"]: _pack_w2(np.asarray(inputs["xcn_w3"], np.float32)),
        nm["w_lin1"]: _pack_w2(np.asarray(inputs["lin_w1"], np.float32)),
        nm["w_lin2"]: _pack_w2(np.asarray(inputs["lin_w2"], np.float32)),
        nm["biases"]: BIA,
    }

    t = np.arange(E_LOC)
    e_loc_of_t = (t % 128) * WEDGE + t // 128
    in_maps = []
    for k in range(NCORES):
        ge = perm[k * E_LOC + e_loc_of_t]
        m = dict(shared)
        m[nm["idx_i"]] = _wrap_idx(ti[ge])
        m[nm["idx_j"]] = _wrap_idx(tj[ge])
        in_maps.append(m)

    kw = dict(trace=True, trace_cores=[0]) if _profile else {}
    res = run_bass_kernel_spmd(nc, in_maps, core_ids=list(range(NCORES)), **kw)
    _CACHE["last"] = res
    y = np.empty(E_TOT, np.float32)
    for k in range(NCORES):
        y[perm[k * E_LOC + np.arange(E_LOC)]] = res.results[k][nm["y_out"]].reshape(-1)
    return y.reshape(E_TOT, 1)
